# revision 5
# baseline (speedup 1.0000x reference)
"""GQA decode attention (b=32, T=4096, 64 q-heads / 8 kv-heads) on 8 trn2 cores.

Tensor-parallel over heads: core i owns kv-head i (q-heads 8i..8i+7),
wqkv block i, KV-cache slice i, wo input-rows 1024i..1024(i+1); bf16
ReduceScatters (pipelined in 4 column-quarters) finish the row-parallel
wo; the host concatenates the shards.

Numerics / layout choices (all validated against the fp32 reference in
numpy before hardware):
  - K/V cache streamed as fp8e3 (e3m4), scaled by 2 host-side; scores
    and PV accumulate in fp32 PSUM. Tokens 0..4094 take this path.
  - Token 4095 (the new rope'd k/v from x) is handled on a separate
    fp16 path: its prob column in the fp8 tiles is zeroed and replaced
    with an exact exp(q.k_new) computed from fp16 SBUF tensors - fp8
    error on that column is exp-amplified (scores there have ~2x the
    std of cache scores) and alone pushes rel-err to 5e-2.
  - wqkv and wo stay fp16: quantizing wqkv to fp8 feeds the same
    amplification through q; wo-fp8 costs 1.2e-2 of the 2e-2 budget.
  - RoPE is linear in q/k at fixed position, so it is folded into the
    wqkv columns host-side (q also absorbs the 1/(2*sqrt(128)) score
    scale; k,v absorb the fp8 cache scale 2).
  - PV is computed transposed (stationary = fp8 V tile, moving = prob
    column) so the attention output lands directly in the [d, head]
    layout wo consumes - no per-batch PE transpose, and the fp8
    stationary gets fast-weight-load.
"""

import math
import sys

import numpy as np

sys.path.insert(0, "/opt/trn_rl_repo")

B = 32          # batch
D = 8192        # model dim
HD = 128        # head dim
H = 8           # q-heads per core
NKV = 8         # kv heads (= cores)
T = 4096        # kv length
NT = T // 128   # t-tiles
KD = D // 128   # k-tiles over model dim
BLK = 1280      # wqkv block per kv head (8*128 q | 128 k | 128 v)
KB = 8          # wqkv k-tiles batched per DMA
KVS = 2.0       # fp8 K/V cache scale
F8MAX = 15.5    # e3m4 max normal
SQ = 512.0      # fp16 storage scale for folded q columns (subnormal guard)
SKV = 32.0      # fp16 storage scale for folded k/v columns
QB = 4          # batches per fused KV DMA
SC = 1024.0     # softmax-denominator pre-scale: keeps 1/den (~1e-6..5e-5)
                # in fp16 normal range; descaled in the phase-3 PSUM copy

_CACHE: dict = {}


def _build():
    from contextlib import ExitStack

    import concourse.tile as tile
    from concourse import bacc, mybir
    from concourse.masks import make_identity

    f32 = mybir.dt.float32
    f16 = mybir.dt.float16
    bf16 = mybir.dt.bfloat16
    f8 = mybir.dt.float8e3
    nc = bacc.Bacc("TRN2", target_bir_lowering=False, debug=False, num_devices=8)

    xT = nc.dram_tensor("xT", [128, KD, B], f16, kind="ExternalInput")
    wq = nc.dram_tensor("wq", [128, KD, BLK], f16, kind="ExternalInput")
    # fused cache: per batch, cols 0:T = 2*K^T [d, t]; cols T:2T = 2*V
    # [t-tile, {128 t-rows x 128 d}] so one 4 MiB DMA covers 4 batches
    kv = nc.dram_tensor("kv", [128, B, 2 * T], f8, kind="ExternalInput")
    woT = nc.dram_tensor("woT", [128, H, D], f16, kind="ExternalInput")
    out_ext = nc.dram_tensor("out", [B // 8, D], bf16, kind="ExternalOutput")

    ExpF = mybir.ActivationFunctionType.Exp
    CopyF = mybir.ActivationFunctionType.Copy

    with tile.TileContext(nc) as tc, ExitStack() as ctx:
        cst = ctx.enter_context(tc.tile_pool(name="const", bufs=1))
        ident = cst.tile([128, 128], f16)
        make_identity(nc, ident[:])
        twos = cst.tile([128, 1], bf16)
        nc.vector.memset(twos[:], 2.0 / SC)
        ones1 = cst.tile([1, 128], f32)
        nc.vector.memset(ones1[:], 1.0)

        wop = ctx.enter_context(tc.tile_pool(name="wo", bufs=11))
        # quad 0 of the fused KV cache gets its own non-phase-1-aliasing
        # pool so its 4 MiB load streams during the wq-bound phase 1
        # instead of waiting for phase-1 SBUF to free (~10 us hole).
        kv0p = ctx.enter_context(tc.tile_pool(name="kv0", bufs=1))
        kv_q0 = kv0p.tile([128, QB, 2 * T], f8)
        qT_sb = cst.tile([128, H, B], f16)      # q~^T  [d, h, b]
        knT_sb = cst.tile([128, B], f16)        # 2*k_new^T [d, b]
        vn_sb = cst.tile([B, HD], f16)          # 2*v_new [b, d]
        vn1_sb = cst.tile([1, B, HD], f16)      # 2*v_new on partition 0
        prn_sb = cst.tile([1, B, H], bf16)      # exp(q.k_new) [1, b, h]
        attT_sb = cst.tile([128, H, B], f16)    # att^T [d, h, b]

        # ---------------- phase 1: fused qkv projection ----------------
        # col-tiled 4-way over k-tiles (M=32 only fills a quarter of the
        # PE array); partial sums per column-group folded by DVE adds.
        with (
            tc.tile_pool(name="w", bufs=2) as wpool,
            tc.tile_pool(name="xt", bufs=1) as xpool,
            tc.tile_pool(name="qps", bufs=1, space="PSUM") as qps,
            tc.tile_pool(name="m1", bufs=1) as m1,
            tc.tile_pool(name="tps", bufs=1, space="PSUM") as tps,
        ):
            xt = xpool.tile([128, KD, B], f16)
            nc.sync.dma_start(xt[:], xT[:])
            # quad 0 streams behind xt on the otherwise-idle sync queue
            nc.sync.dma_start(kv_q0[:], kv[:, 0:QB, :])
            ps_all = qps.tile([128, BLK], f32)
            NR = KD // 4  # accumulation rounds per column-group
            for kk in range(0, KD, KB):
                wt = wpool.tile([128, KB, BLK], f16)
                nc.scalar.dma_start(wt[:], wq[:, kk:kk + KB, :])
                for k in range(KB):
                    g, r = (kk + k) % 4, (kk + k) // 4
                    lhs = xt[:, kk + k, :]
                    st, sp = r == 0, r == NR - 1
                    po = 32 * g
                    tp = (0, po)
                    nc.tensor.matmul(ps_all[po:po + 32, 0:512], lhs,
                                     wt[:, k, 0:512], start=st, stop=sp,
                                     tile_position=tp)
                    nc.tensor.matmul(ps_all[po:po + 32, 512:1024], lhs,
                                     wt[:, k, 512:1024], start=st, stop=sp,
                                     tile_position=tp)
                    nc.tensor.matmul(ps_all[po:po + 32, 1024:1280], lhs,
                                     wt[:, k, 1024:1280], start=st, stop=sp,
                                     tile_position=tp)

            # fold the 4 column-group partials (DVE reads at most one PSUM
            # operand per instruction, so chain through SBUF)
            s0 = m1.tile([32, BLK], f32)
            nc.scalar.activation(s0[:], ps_all[0:32, :], CopyF)
            s1 = m1.tile([32, BLK], f32)
            nc.vector.tensor_add(s1[:], s0[:], ps_all[32:64, :])
            s2 = m1.tile([32, BLK], f32)
            nc.vector.tensor_add(s2[:], s1[:], ps_all[64:96, :])
            qkv = m1.tile([32, BLK], f16)
            nc.vector.tensor_add(qkv[:], s2[:], ps_all[96:128, :])

            t_ps = tps.tile([128, H, B], f16)
            for h in range(H):
                nc.tensor.transpose(
                    t_ps[:, h, :], qkv[:, h * 128:(h + 1) * 128], ident[0:B, 0:B]
                )
            nc.scalar.activation(qT_sb[:], t_ps[:], CopyF, scale=1.0 / SQ)
            t2_ps = tps.tile([128, B], f16)
            nc.tensor.transpose(t2_ps[:], qkv[:, 1024:1152], ident[0:B, 0:B])
            nc.scalar.activation(knT_sb[:], t2_ps[:], CopyF, scale=1.0 / SKV)
            nc.scalar.activation(vn_sb[:], qkv[:, 1152:1280], CopyF, scale=1.0 / SKV)
            # flatten v_new onto partition 0 so the per-batch rank-1 PV
            # update has a {0}-based stationary operand. On gpsimd: the sync
            # queue must stay clear so phase 2's first KV loads issue during
            # phase 1.
            nc.gpsimd.dma_start(vn1_sb[:], vn_sb[:])

            # fp16 side path for token 4095: scn[b, h] = q~_b . (2 k_new_b)
            psn = tps.tile([1, B, H], f32)
            for b in range(B):
                nc.tensor.matmul(psn[:, b, :], knT_sb[:, b:b + 1],
                                 qT_sb[:, :, b], start=True, stop=True)
            nc.scalar.activation(prn_sb[:], psn[:], ExpF)

        # ---------------- phase 2: attention over batches ----------------
        with (
            tc.tile_pool(name="kv", bufs=2) as kvp,
            tc.tile_pool(name="pr", bufs=3) as prp,
            tc.tile_pool(name="scps", bufs=2, space="PSUM") as scp,
            tc.tile_pool(name="ovps", bufs=2, space="PSUM") as ovp,
            tc.tile_pool(name="dnps", bufs=2, space="PSUM") as dnp,
            tc.tile_pool(name="dbps", bufs=2, space="PSUM") as dbp,
            tc.tile_pool(name="att", bufs=4) as attp,
        ):
            wt_tiles = []
            kv_quad = kv_q0
            for b in range(B):
                if b % QB == 0 and b > 0:
                    kv_quad = kvp.tile([128, QB, 2 * T], f8, name="kvq", tag="kvq")
                    nc.sync.dma_start(kv_quad[:], kv[:, b:b + QB, :])
                if b % 2 == 0:
                    # paced prefetch of wo weight tiles (11 of 16 here)
                    if b >= 2 and len(wt_tiles) < 11:
                        i = len(wt_tiles)
                        half, k = divmod(i, H)
                        wt = wop.tile([128, 4096], f16, name="wt", tag="wt")
                        nc.scalar.dma_start(
                            wt[:], woT[:, k, half * 4096:(half + 1) * 4096])
                        wt_tiles.append(wt)
                kt_t = kv_quad[:, b % QB, 0:T]
                vt_t = kv_quad[:, b % QB, T:2 * T]

                sc = scp.tile([128, NT, H], f32)
                for j in range(NT):
                    nc.tensor.matmul(
                        sc[:, j, :], kt_t[:, j * 128:(j + 1) * 128], qT_sb[:, :, b],
                        start=True, stop=True,
                    )
                pr = prp.tile([128, NT, H], bf16)
                nc.scalar.activation(pr[:], sc[:], ExpF)

                # denominator: collapse partitions with a twos-stationary
                # matmul (V carries the 2x cache scale), fold the 32 tiles
                # with a strided DVE reduce, add the side-path term.
                # pr[127, NT-1] is token 4095's stale-cache prob: excluded
                # here and in PV, replaced by the fp16 side path.
                dn1 = dnp.tile([1, NT * H], f32)
                nc.tensor.matmul(dn1[:, 0:(NT - 1) * H], twos[:],
                                 pr[:, 0:NT - 1, :], start=True, stop=True)
                nc.tensor.matmul(dn1[:, (NT - 1) * H:], twos[0:127, :],
                                 pr[0:127, NT - 1, :], start=True, stop=True)
                den8 = attp.tile([1, H], f32, name="den8", tag="den8")
                nc.vector.reduce_sum(
                    den8[:], dn1.rearrange("p (t h) -> p h t", h=H),
                    axis=mybir.AxisListType.X,
                )
                pn2 = attp.tile([1, H], f32, name="pn2", tag="pn2")
                nc.vector.tensor_scalar_mul(pn2[:], prn_sb[:, b, :], 2.0 / SC)
                denb = attp.tile([1, H], f32, name="denb", tag="denb")
                nc.vector.tensor_add(denb[:], den8[:], pn2[:])
                rec8 = attp.tile([1, H], f32, name="rec8", tag="rec8")
                nc.vector.reciprocal(rec8[:], denb[:])

                # PV transposed: out[d, h] accumulated over t-tiles, plus the
                # fp16 rank-1 update for token 4095.
                ov = ovp.tile([128, H], f32)
                for j in range(NT - 1):
                    nc.tensor.matmul(
                        ov[:], vt_t[:, j * HD:(j + 1) * HD], pr[:, j, :],
                        start=(j == 0), stop=False,
                    )
                nc.tensor.matmul(
                    ov[:], vt_t[0:127, (NT - 1) * HD:NT * HD],
                    pr[0:127, NT - 1, :], start=False, stop=False,
                )
                nc.tensor.matmul(
                    ov[:], vn1_sb[:, b, :], prn_sb[:, b, :],
                    start=False, stop=True,
                )
                # broadcast 1/den over partitions with a 1-row outer product
                dbc = dbp.tile([128, H], f32)
                nc.tensor.matmul(dbc[:], ones1[:], rec8[:], start=True, stop=True)
                dbc_sb = attp.tile([128, H], f16, name="dbc", tag="dbc")
                nc.scalar.activation(dbc_sb[:], dbc[:], CopyF)
                nc.vector.tensor_mul(attT_sb[:, :, b], ov[:], dbc_sb[:])

        # ------- phase 3: wo row-parallel, 2 column-halves + RS pipeline -------
        with (
            tc.tile_pool(name="wops", bufs=1, space="PSUM") as wops,
            tc.tile_pool(name="ob", bufs=2) as obp,
            tc.tile_pool(name="dram", bufs=1, space="DRAM") as dram,
        ):
            for half in range(2):
                off = half * 4096
                psw = wops.tile([32, 4096], f32, name="psw", tag="psw")
                for k in range(H):
                    i = half * H + k
                    if i < len(wt_tiles):
                        wt = wt_tiles[i]
                    else:
                        wt = wop.tile([128, 4096], f16, name="wt", tag="wt")
                        nc.scalar.dma_start(
                            wt[:], woT[:, k, half * 4096:(half + 1) * 4096]
                        )
                        wt_tiles.append(wt)
                    for n in range(8):
                        nc.tensor.matmul(
                            psw[:, n * 512:(n + 1) * 512], attT_sb[:, k, :],
                            wt[:, n * 512:(n + 1) * 512],
                            start=(k == 0), stop=(k == H - 1),
                        )
                ob = obp.tile([B, 4096], bf16, name="ob", tag="ob")
                nc.vector.tensor_scalar_mul(ob[:], psw[:], 1.0 / SC)
                cc_in = dram.tile([B, 4096], bf16, name=f"cc_in{half}")
                cc_out = dram.tile([B // 8, 4096], bf16, name=f"cc_out{half}")
                nc.sync.dma_start(cc_in[:], ob[:])
                nc.gpsimd.collective_compute(
                    "ReduceScatter",
                    mybir.AluOpType.add,
                    replica_groups=[list(range(8))],
                    ins=[cc_in.opt()],
                    outs=[cc_out.opt()],
                )
                nc.scalar.dma_start(out_ext[:, off:off + 4096], cc_out[:])

    nc.compile()
    return nc


def _prep_inputs(x, cache_k, cache_v, wqkv_w, wo_w, freqs_cos, freqs_sin):
    import ml_dtypes

    f16 = np.float16
    f8 = ml_dtypes.float8_e3m4
    cos = np.asarray(freqs_cos, np.float32).reshape(-1)[:64]
    sin = np.asarray(freqs_sin, np.float32).reshape(-1)[:64]
    x = np.asarray(x, np.float32).reshape(B, D)
    # x^T packed tile-major: xT[p, k, b] = x[b, 128k+p]
    xT = np.ascontiguousarray(x.reshape(B, KD, 128).transpose(2, 1, 0)).astype(f16)

    wqkv_w = np.asarray(wqkv_w, np.float32)
    scale = SQ / (KVS * math.sqrt(HD))
    in_maps = []
    for c in range(8):
        W = wqkv_w[:, c * BLK:(c + 1) * BLK].copy()
        q = W[:, :1024].reshape(D, H, 64, 2)
        q0 = q[..., 0].copy()
        q1 = q[..., 1].copy()
        q[..., 0] = (q0 * cos - q1 * sin) * scale
        q[..., 1] = (q0 * sin + q1 * cos) * scale
        k = W[:, 1024:1152].reshape(D, 64, 2)
        k0 = k[..., 0].copy()
        k1 = k[..., 1].copy()
        k[..., 0] = (k0 * cos - k1 * sin) * (KVS * SKV)
        k[..., 1] = (k0 * sin + k1 * cos) * (KVS * SKV)
        W[:, 1152:1280] *= KVS * SKV
        # partition-major: wq_pm[p, kt, :] = W[kt*128+p, :]
        W_pm = np.ascontiguousarray(
            W.reshape(KD, 128, BLK).transpose(1, 0, 2)
        ).astype(f16)

        kvc = np.empty((128, B, 2 * T), np.float32)
        kvc[:, :, 0:T] = np.asarray(
            cache_k[:, :, c, :], np.float32).transpose(2, 0, 1)  # [128(d), B, T]
        kvc[:, :, T:2 * T] = (
            np.asarray(cache_v[:, :, c, :], np.float32)
            .reshape(B, NT, 128, HD)
            .transpose(2, 0, 1, 3)              # [128(t%128), B, NT, HD]
            .reshape(128, B, T)
        )
        kvc = np.clip(kvc * KVS, -F8MAX, F8MAX).astype(f8)
        woTc = np.asarray(wo_w[:, c * 1024:(c + 1) * 1024], np.float32).T  # [1024, D]
        woT_pm = np.ascontiguousarray(
            woTc.reshape(H, 128, D).transpose(1, 0, 2)
        ).astype(f16)
        in_maps.append({
            "xT": xT, "wq": W_pm, "kv": kvc, "woT": woT_pm,
        })
    return in_maps


def kernel(x, cache_k, cache_v, wqkv_w, wo_w, freqs_cos, freqs_sin, mask,
           start_pos, _want_trace=False, **_unused):
    from concourse.bass_utils import run_bass_kernel_spmd

    sp = int(np.asarray(start_pos))
    assert sp == T - 1, f"kernel compiled for start_pos={T - 1}, got {sp}"

    if "nc" not in _CACHE:
        _CACHE["nc"] = _build()
    nc = _CACHE["nc"]

    in_maps = _prep_inputs(x, cache_k, cache_v, wqkv_w, wo_w, freqs_cos, freqs_sin)
    res = run_bass_kernel_spmd(nc, in_maps, list(range(8)), trace=_want_trace)
    # ReduceScatter leaves rank i holding reduced rows 4i..4(i+1): concatenate
    out = np.concatenate(
        [np.asarray(res.results[i]["out"]) for i in range(8)], axis=0
    ).astype(np.float32)
    out = out.reshape(B, 1, D)
    if _want_trace:
        _CACHE["last_result"] = res
    return out
